# revision 4
# baseline (speedup 1.0000x reference)
"""HMM forward-backward (batch=256, seq=512, Z=64) on 8 Trainium2 NeuronCores.

Strategy (data parallel over batch, 32 batch elements per core):
  - Emission rows e[t,b,:] = emit[input[t,b]] are gathered on-device with
    dma_gather (int16 indices), landing in natural [row=(t,b), Z] layout,
    then PE-transposed to a [Z, (t,b)] layout ("E2").
  - Forward and backward recursions are merged into ONE 128-contraction
    matmul per step with a block-diagonal stationary matrix
    W = diag(T, T^T):  state = [v_{S-1-q} (rows 0:64) ; alpha_q (rows 64:128)]
    per column group q.  One DVE multiply with the E2 column produces the
    next state column.  beta_{S-2-j} is the top PSUM half before the
    multiply and is copied off by the Scalar engine.
  - posterior = (alpha*beta) normalized per (t,b) over Z: Z lives on the
    partition dim, so the column sums use a ones-vector matmul, then
    reciprocal + gpsimd partition_broadcast + DVE multiplies.
  - Outputs are produced in [Z, t*32+b] layout per core; the host
    reassembles/transposes to [S, B, Z] (pure numpy layout work).
"""

import sys

for _p in ("/opt/trn_rl_repo", "/root/.axon_site/_ro/trn_rl_repo"):
    if _p not in sys.path:
        sys.path.append(_p)

import numpy as np

import concourse.bacc as bacc
import concourse.mybir as mybir
from concourse.bass_utils import run_bass_kernel_spmd
from concourse.tile import TileContext

S = 512          # sequence length
B = 256          # total batch
Z = 64           # hidden states
X = 10000        # emission vocab
NCORES = 8
Bc = B // NCORES           # batch per core = 32
COLS = S * Bc              # 16384 state columns per core
CH = 64                    # timesteps per gather/E2 chunk
CCOLS = CH * Bc            # 2048 columns per chunk
NCH = S // CH              # 8 chunks
NBLK = CCOLS // 128        # 16 transpose blocks (of 4 timesteps) per chunk
PCOLS = 1024               # posterior chunk columns (32 timesteps)
NPCH = COLS // PCOLS       # 16 posterior chunks

F32 = mybir.dt.float32
MUL = mybir.AluOpType.mult

_CACHE = {}
LAST_RESULTS = None


def _build_nc():
    nc = bacc.Bacc("TRN2", target_bir_lowering=False, debug=False,
                   num_devices=NCORES)

    emit_d = nc.dram_tensor("emit", [X, Z], F32, kind="ExternalInput")
    idxc_d = nc.dram_tensor("idxc", [128, 2 * COLS // 16], mybir.dt.int16,
                            kind="ExternalInput")
    w_d = nc.dram_tensor("w", [128, 128], F32, kind="ExternalInput")
    id_d = nc.dram_tensor("ident", [128, 128], F32, kind="ExternalInput")
    pi_d = nc.dram_tensor("piext", [128, 1], F32, kind="ExternalInput")

    alpha_d = nc.dram_tensor("alpha", [64, COLS], F32, kind="ExternalOutput")
    beta_d = nc.dram_tensor("beta", [64, COLS], F32, kind="ExternalOutput")
    post_d = nc.dram_tensor("post", [64, COLS], F32, kind="ExternalOutput")

    with TileContext(nc) as tc:
        with (
            tc.tile_pool(name="const", bufs=1) as constp,
            tc.tile_pool(name="state", bufs=1) as statep,
            tc.tile_pool(name="betap", bufs=1) as betapp,
            tc.tile_pool(name="e2", bufs=2) as e2p,
            tc.tile_pool(name="gst", bufs=3) as gstp,
            tc.tile_pool(name="ab", bufs=2) as abp,
            tc.tile_pool(name="bc", bufs=2) as bcp,
            tc.tile_pool(name="po", bufs=2) as pop,
            tc.tile_pool(name="rec", bufs=1) as recp,
            tc.tile_pool(name="mm", bufs=4, space="PSUM") as mmp,
            tc.tile_pool(name="tr", bufs=3, space="PSUM") as trp,
        ):
            # ---- constants ----
            idxc_t = constp.tile([128, 2 * COLS // 16], mybir.dt.int16,
                                 tag="idxc")
            w_t = constp.tile([128, 128], F32, tag="w")
            id_t = constp.tile([128, 128], F32, tag="id")
            pi_t = constp.tile([128, 1], F32, tag="pi")
            nc.sync.dma_start(idxc_t[:], idxc_d[:])
            nc.sync.dma_start(w_t[:], w_d[:])
            nc.sync.dma_start(id_t[:], id_d[:])
            nc.sync.dma_start(pi_t[:], pi_d[:])

            state = statep.tile([128, COLS], F32, tag="state")
            betap = betapp.tile([128, COLS], F32, tag="beta")  # rows 64:128

            gtiles = {}   # (chunk, which 0=bwd/top 1=fwd/bottom) -> tile
            e2tiles = {}  # chunk -> tile

            # dma_gather is limited to ~1024 indices per instruction.
            # One interleaved gather per chunk: even blocks = bwd rows,
            # odd blocks = fwd rows -> [128, 2*NBLK, 64] staging.
            GI = 1024
            GSPLIT = 2 * CCOLS // GI  # 4 gathers per chunk

            def issue_gather(c):
                g = gstp.tile([128, 2 * NBLK, Z], F32, tag="g", name=f"g_{c}")
                for h in range(GSPLIT):
                    nb = GI // 128
                    nc.gpsimd.dma_gather(
                        g[:, h * nb:(h + 1) * nb, :], emit_d[:],
                        idxc_t[:, c * (2 * CCOLS // 16) + h * (GI // 16):
                               c * (2 * CCOLS // 16) + (h + 1) * (GI // 16)],
                        GI, GI, Z)
                gtiles[c] = g

            def transpose_pair(c, m):
                if m == 0:
                    e2tiles[c] = e2p.tile([128, CCOLS], F32, tag="e2", name=f"e2_{c}")
                e2 = e2tiles[c]
                pA = trp.tile([128, 128], F32, tag="tr")
                gv = gtiles[c][:, 2 * m:2 * m + 2, :]
                nc.tensor.transpose(pA[:], gv.rearrange("p a b -> p (a b)"),
                                    id_t[:])
                cs = slice(m * 128, (m + 1) * 128)
                nc.scalar.copy(e2[:, cs], pA[:])

            # ---- prologue ----
            issue_gather(0)
            issue_gather(1)
            for m in range(NBLK):
                transpose_pair(0, m)
            # state col 0 = E2 col 0 * [ones; pi]
            nc.vector.tensor_scalar(state[:, 0:Bc], e2tiles[0][:, 0:Bc],
                                    pi_t[:, 0:1], None, MUL)
            # beta[S-1] = 1
            nc.vector.memset(betap[64:128, (S - 1) * Bc:S * Bc], 1.0)

            # transpose emission schedule: pair m of chunk c+1 is emitted
            # inside chunk c at step offset TR_SLOTS[m]
            TR_SLOTS = {4 + 3 * m: m for m in range(NBLK)}

            # ---- merged forward/backward scan ----
            for j in range(S - 1):
                c, off = j // CH, j % CH
                if off == 0 and c + 2 < NCH:
                    issue_gather(c + 2)
                if off in TR_SLOTS and c + 1 < NCH:
                    transpose_pair(c + 1, TR_SLOTS[off])

                ps = mmp.tile([128, Bc], F32, tag="mm")
                nc.tensor.matmul(ps[:], w_t[:], state[:, j * Bc:(j + 1) * Bc])
                q = j + 1
                nc.vector.tensor_tensor(
                    state[:, q * Bc:(q + 1) * Bc], ps[:],
                    e2tiles[q // CH][:, (q % CH) * Bc:((q % CH) + 1) * Bc],
                    MUL)
                tb = S - 2 - j
                nc.scalar.copy(betap[64:128, tb * Bc:(tb + 1) * Bc],
                               ps[0:64, :])

            # ---- outputs: alpha/beta straight out, posterior normalized ----
            for k in range(4):
                cs = slice(k * (COLS // 4), (k + 1) * (COLS // 4))
                nc.sync.dma_start(alpha_d[:, cs], state[64:128, cs])
                nc.sync.dma_start(beta_d[:, cs], betap[64:128, cs])

            for p in range(NPCH):
                cs = slice(p * PCOLS, (p + 1) * PCOLS)
                ab = abp.tile([64, PCOLS], F32, tag="ab")
                nc.vector.tensor_tensor(ab[:], state[64:128, cs],
                                        betap[64:128, cs], MUL)
                rec = recp.tile([1, PCOLS], F32, tag="rec")
                for h in range(2):
                    pssum = trp.tile([1, 512], F32, tag="tr")
                    nc.tensor.matmul(pssum[:], pi_t[0:64, 0:1],
                                     ab[:, h * 512:(h + 1) * 512])
                    nc.vector.reciprocal(rec[:, h * 512:(h + 1) * 512],
                                         pssum[:])
                bct = bcp.tile([64, PCOLS], F32, tag="bc")
                nc.gpsimd.partition_broadcast(bct[:], rec[:, :])
                po = pop.tile([64, PCOLS], F32, tag="po")
                nc.vector.tensor_tensor(po[:], ab[:], bct[:], MUL)
                nc.sync.dma_start(post_d[:, cs], po[:])

    nc.finalize()
    return nc


def _wrap_idx(lin):
    """Linear index list -> [128, N//16] int16 gather-index layout
    (position i at [i % 16, i // 16], replicated over partition groups)."""
    n = lin.shape[0]
    w = lin.reshape(n // 16, 16).T.astype(np.int16)   # [16, n//16]
    return np.tile(w, (8, 1))


def kernel(input, T, pi, emit):
    global LAST_RESULTS
    input = np.asarray(input)
    T = np.asarray(T, dtype=np.float32)
    pi = np.asarray(pi, dtype=np.float32)
    emit = np.asarray(emit, dtype=np.float32)

    if "nc" not in _CACHE:
        _CACHE["nc"] = _build_nc()
    nc = _CACHE["nc"]

    W = np.zeros((128, 128), np.float32)
    W[:64, :64] = T          # backward block: out_top = T^T @ v
    W[64:, 64:] = T.T        # forward block:  out_bot = T @ alpha
    pi_ext = np.ones((128, 1), np.float32)
    pi_ext[64:, 0] = pi
    ident = np.eye(128, dtype=np.float32)

    in_maps = []
    for c in range(NCORES):
        sl = input[:, c * Bc:(c + 1) * Bc].astype(np.int64)   # [S, Bc]
        lin_f = sl.reshape(-1)                                # i = t*Bc+b
        lin_b = sl[::-1, :].reshape(-1)                       # i = k*Bc+b, t=S-1-k
        # interleave 128-row blocks: [bwd m, fwd m] per block pair
        fb = lin_b.reshape(-1, 128)                           # [128 blocks, 128]
        ff = lin_f.reshape(-1, 128)
        lin_c = np.stack([fb, ff], axis=1).reshape(-1)        # [2*COLS]
        in_maps.append({
            "emit": emit,
            "idxc": _wrap_idx(lin_c),
            "w": W,
            "ident": ident,
            "piext": pi_ext,
        })

    res = run_bass_kernel_spmd(nc, in_maps, core_ids=list(range(NCORES)))
    LAST_RESULTS = res

    alpha = np.empty((S, B, Z), np.float32)
    beta = np.empty((S, B, Z), np.float32)
    post = np.empty((S, B, Z), np.float32)
    for c in range(NCORES):
        r = res.results[c]
        bs = slice(c * Bc, (c + 1) * Bc)
        alpha[:, bs, :] = r["alpha"].reshape(Z, S, Bc).transpose(1, 2, 0)
        beta[:, bs, :] = r["beta"].reshape(Z, S, Bc).transpose(1, 2, 0)
        post[:, bs, :] = r["post"].reshape(Z, S, Bc).transpose(1, 2, 0)
    return alpha, beta, post


# revision 6
# speedup vs baseline: 1.0654x; 1.0654x over previous
"""HMM forward-backward (batch=256, seq=512, Z=64) on 8 Trainium2 NeuronCores.

Strategy (data parallel over batch, 32 batch elements per core):
  - Emission rows e[t,b,:] = emit[input[t,b]] are gathered on-device with
    dma_gather (int16 indices), landing in natural [row=(t,b), Z] layout,
    then PE-transposed to a [Z, (t,b)] layout ("E2").
  - Forward and backward recursions are merged into ONE 128-contraction
    matmul per step with a block-diagonal stationary matrix
    W = diag(T, T^T):  state = [v_{S-1-q} (rows 0:64) ; alpha_q (rows 64:128)]
    per column group q.  One DVE multiply with the E2 column produces the
    next state column.  beta_{S-2-j} is the top PSUM half before the
    multiply and is copied off by the Scalar engine.
  - posterior = (alpha*beta) normalized per (t,b) over Z: Z lives on the
    partition dim, so the column sums use a ones-vector matmul, then
    reciprocal + gpsimd partition_broadcast + DVE multiplies.
  - Outputs are produced in [Z, t*32+b] layout per core; the host
    reassembles/transposes to [S, B, Z] (pure numpy layout work).
"""

import sys

for _p in ("/opt/trn_rl_repo", "/root/.axon_site/_ro/trn_rl_repo"):
    if _p not in sys.path:
        sys.path.append(_p)

import numpy as np

import concourse.bacc as bacc
import concourse.mybir as mybir
from concourse.bass_utils import run_bass_kernel_spmd
from concourse.tile import TileContext

S = 512          # sequence length
B = 256          # total batch
Z = 64           # hidden states
X = 10000        # emission vocab
NCORES = 8
Bc = B // NCORES           # batch per core = 32
COLS = S * Bc              # 16384 state columns per core
CH = 64                    # timesteps per gather/E2 chunk
CCOLS = CH * Bc            # 2048 columns per chunk
NCH = S // CH              # 8 chunks
NBLK = CCOLS // 128        # 16 transpose blocks (of 4 timesteps) per chunk
PCOLS = 1024               # posterior chunk columns (32 timesteps)
NPCH = COLS // PCOLS       # 16 posterior chunks

F32 = mybir.dt.float32
MUL = mybir.AluOpType.mult

_CACHE = {}
LAST_RESULTS = None


def _build_nc():
    nc = bacc.Bacc("TRN2", target_bir_lowering=False, debug=False,
                   num_devices=NCORES)

    emit_d = nc.dram_tensor("emit", [X, Z], F32, kind="ExternalInput")
    idxc_d = nc.dram_tensor("idxc", [128, 2 * COLS // 16], mybir.dt.int16,
                            kind="ExternalInput")
    w_d = nc.dram_tensor("w", [128, 128], F32, kind="ExternalInput")
    id_d = nc.dram_tensor("ident", [128, 128], F32, kind="ExternalInput")
    pi_d = nc.dram_tensor("piext", [128, 1], F32, kind="ExternalInput")

    alpha_d = nc.dram_tensor("alpha", [64, COLS], F32, kind="ExternalOutput")
    beta_d = nc.dram_tensor("beta", [64, COLS], F32, kind="ExternalOutput")
    post_d = nc.dram_tensor("post", [64, COLS], F32, kind="ExternalOutput")

    with TileContext(nc) as tc:
        with (
            tc.tile_pool(name="const", bufs=1) as constp,
            tc.tile_pool(name="state", bufs=1) as statep,
            tc.tile_pool(name="betap", bufs=1) as betapp,
            tc.tile_pool(name="e2", bufs=2) as e2p,
            tc.tile_pool(name="gst", bufs=3) as gstp,
            tc.tile_pool(name="ab", bufs=2) as abp,
            tc.tile_pool(name="bc", bufs=2) as bcp,
            tc.tile_pool(name="po", bufs=2) as pop,
            tc.tile_pool(name="rec", bufs=1) as recp,
            tc.tile_pool(name="mm", bufs=4, space="PSUM") as mmp,
            tc.tile_pool(name="tr", bufs=3, space="PSUM") as trp,
        ):
            # ---- constants ----
            idxc_t = constp.tile([128, 2 * COLS // 16], mybir.dt.int16,
                                 tag="idxc")
            w_t = constp.tile([128, 128], F32, tag="w")
            id_t = constp.tile([128, 128], F32, tag="id")
            pi_t = constp.tile([128, 1], F32, tag="pi")
            nc.sync.dma_start(idxc_t[:], idxc_d[:])
            nc.sync.dma_start(w_t[:], w_d[:])
            nc.sync.dma_start(id_t[:], id_d[:])
            nc.sync.dma_start(pi_t[:], pi_d[:])

            state = statep.tile([128, COLS], F32, tag="state")
            betap = betapp.tile([128, COLS], F32, tag="beta")  # rows 64:128

            gtiles = {}   # (chunk, which 0=bwd/top 1=fwd/bottom) -> tile
            e2tiles = {}  # chunk -> tile

            # dma_gather is limited to ~1024 indices per instruction.
            # One interleaved gather per chunk: even blocks = bwd rows,
            # odd blocks = fwd rows -> [128, 2*NBLK, 64] staging.
            GI = 1024
            GSPLIT = 2 * CCOLS // GI  # 4 gathers per chunk

            def issue_gather(c):
                g = gstp.tile([128, 2 * NBLK, Z], F32, tag="g", name=f"g_{c}")
                for h in range(GSPLIT):
                    nb = GI // 128
                    nc.gpsimd.dma_gather(
                        g[:, h * nb:(h + 1) * nb, :], emit_d[:],
                        idxc_t[:, c * (2 * CCOLS // 16) + h * (GI // 16):
                               c * (2 * CCOLS // 16) + (h + 1) * (GI // 16)],
                        GI, GI, Z)
                gtiles[c] = g

            def transpose_pair(c, m):
                if m == 0:
                    e2tiles[c] = e2p.tile([128, CCOLS], F32, tag="e2", name=f"e2_{c}")
                e2 = e2tiles[c]
                pA = trp.tile([128, 128], F32, tag="tr")
                gv = gtiles[c][:, 2 * m:2 * m + 2, :]
                nc.tensor.transpose(pA[:], gv.rearrange("p a b -> p (a b)"),
                                    id_t[:])
                cs = slice(m * 128, (m + 1) * 128)
                nc.scalar.copy(e2[:, cs], pA[:])

            # ---- prologue ----
            issue_gather(0)
            issue_gather(1)
            for m in range(NBLK):
                transpose_pair(0, m)
            # state col 0 = E2 col 0 * [ones; pi]
            nc.vector.tensor_scalar(state[:, 0:Bc], e2tiles[0][:, 0:Bc],
                                    pi_t[:, 0:1], None, MUL)
            # beta[S-1] = 1
            nc.vector.memset(betap[64:128, (S - 1) * Bc:S * Bc], 1.0)

            # ---- posterior machinery: sliced ops so they fit engine idle
            # windows inside the scan without stretching the chain ----
            PSL = 256                       # posterior slice columns
            NSL = PCOLS // PSL              # 4 slices per chunk

            def post_ops(p):
                """Closures computing posterior chunk p, in dependency order,
                each small enough to hide in per-step engine idle time."""
                cs = slice(p * PCOLS, (p + 1) * PCOLS)
                ctx = {}
                ops = []

                def mk_ab(k):
                    def fn():
                        if k == 0:
                            ctx["ab"] = abp.tile([64, PCOLS], F32, tag="ab",
                                                 name=f"ab_{p}")
                        s = slice(p * PCOLS + k * PSL, p * PCOLS + (k + 1) * PSL)
                        nc.vector.tensor_tensor(
                            ctx["ab"][:, k * PSL:(k + 1) * PSL],
                            state[64:128, s], betap[64:128, s], MUL)
                    return fn

                def mk_sum(k):
                    def fn():
                        if k == 0:
                            ctx["rec"] = recp.tile([1, PCOLS], F32, tag="rec",
                                                   name=f"rec_{p}")
                        pssum = trp.tile([1, PSL], F32, tag="tr",
                                         name=f"pss_{p}_{k}")
                        nc.tensor.matmul(pssum[:], pi_t[0:64, 0:1],
                                         ctx["ab"][:, k * PSL:(k + 1) * PSL])
                        nc.vector.reciprocal(
                            ctx["rec"][:, k * PSL:(k + 1) * PSL], pssum[:])
                    return fn

                def mk_bcast():
                    def fn():
                        ctx["bc"] = bcp.tile([64, PCOLS], F32, tag="bc",
                                             name=f"bc_{p}")
                        nc.gpsimd.partition_broadcast(ctx["bc"][:],
                                                      ctx["rec"][:, :])
                    return fn

                def mk_pm(k):
                    def fn():
                        if k == 0:
                            ctx["po"] = pop.tile([64, PCOLS], F32, tag="po",
                                                 name=f"po_{p}")
                        nc.vector.tensor_tensor(
                            ctx["po"][:, k * PSL:(k + 1) * PSL],
                            ctx["ab"][:, k * PSL:(k + 1) * PSL],
                            ctx["bc"][:, k * PSL:(k + 1) * PSL], MUL)
                        if k == NSL - 1:
                            nc.sync.dma_start(post_d[:, cs], ctx["po"][:])
                    return fn

                for k in range(NSL):
                    ops.append(mk_ab(k))
                for k in range(NSL):
                    ops.append(mk_sum(k))
                ops.append(mk_bcast())
                for k in range(NSL):
                    ops.append(mk_pm(k))
                return ops

            # chunks whose alpha+beta are ready mid-scan: p=8..14 inline,
            # two ops every step starting 2 steps after alpha lands
            INLINE_PCH = list(range(8, 15))
            TAIL_PCH = [15] + list(range(0, 8))
            POST_SCHED = {}
            for p in INLINE_PCH:
                ops = post_ops(p)
                j0 = 32 * p + 33
                for i, fn in enumerate(ops):
                    POST_SCHED.setdefault(j0 + 2 * i, []).append(fn)

            # transpose emission schedule: pair m of chunk c+1 is emitted
            # inside chunk c at step offset TR_SLOTS[m]
            TR_SLOTS = {4 + 3 * m: m for m in range(NBLK)}

            # ---- merged forward/backward scan ----
            for j in range(S - 1):
                c, off = j // CH, j % CH
                if off == 0 and c + 2 < NCH:
                    issue_gather(c + 2)
                if off in TR_SLOTS and c + 1 < NCH:
                    transpose_pair(c + 1, TR_SLOTS[off])

                ps = mmp.tile([128, Bc], F32, tag="mm")
                nc.tensor.matmul(ps[:], w_t[:], state[:, j * Bc:(j + 1) * Bc])
                q = j + 1
                nc.vector.tensor_tensor(
                    state[:, q * Bc:(q + 1) * Bc], ps[:],
                    e2tiles[q // CH][:, (q % CH) * Bc:((q % CH) + 1) * Bc],
                    MUL)
                tb = S - 2 - j
                nc.scalar.copy(betap[64:128, tb * Bc:(tb + 1) * Bc],
                               ps[0:64, :])
                # stream completed output chunks out under the scan
                if off == CH - 1 and c < NCH - 1:       # alpha chunk c done
                    acs = slice(c * CCOLS, (c + 1) * CCOLS)
                    nc.sync.dma_start(alpha_d[:, acs], state[64:128, acs])
                bc_ = (S - 1 - j) // CH                 # beta chunk bc_ done when j == 511-64*bc_
                if bc_ >= 1 and j == (S - 1) - CH * bc_ and bc_ <= NCH - 1:
                    bcs = slice(bc_ * CCOLS, (bc_ + 1) * CCOLS)
                    nc.sync.dma_start(beta_d[:, bcs], betap[64:128, bcs])
                for fn in POST_SCHED.get(j, []):
                    fn()

            # ---- outputs: alpha/beta straight out, posterior normalized ----
            cs = slice((NCH - 1) * CCOLS, NCH * CCOLS)
            nc.sync.dma_start(alpha_d[:, cs], state[64:128, cs])
            cs = slice(0, CCOLS)
            nc.sync.dma_start(beta_d[:, cs], betap[64:128, cs])

            for p in TAIL_PCH:
                for fn in post_ops(p):
                    fn()

    nc.finalize()
    return nc


def _wrap_idx(lin):
    """Linear index list -> [128, N//16] int16 gather-index layout
    (position i at [i % 16, i // 16], replicated over partition groups)."""
    n = lin.shape[0]
    w = lin.reshape(n // 16, 16).T.astype(np.int16)   # [16, n//16]
    return np.tile(w, (8, 1))


def kernel(input, T, pi, emit):
    global LAST_RESULTS
    input = np.asarray(input)
    T = np.asarray(T, dtype=np.float32)
    pi = np.asarray(pi, dtype=np.float32)
    emit = np.asarray(emit, dtype=np.float32)

    if "nc" not in _CACHE:
        _CACHE["nc"] = _build_nc()
    nc = _CACHE["nc"]

    W = np.zeros((128, 128), np.float32)
    W[:64, :64] = T          # backward block: out_top = T^T @ v
    W[64:, 64:] = T.T        # forward block:  out_bot = T @ alpha
    pi_ext = np.ones((128, 1), np.float32)
    pi_ext[64:, 0] = pi
    ident = np.eye(128, dtype=np.float32)

    in_maps = []
    for c in range(NCORES):
        sl = input[:, c * Bc:(c + 1) * Bc].astype(np.int64)   # [S, Bc]
        lin_f = sl.reshape(-1)                                # i = t*Bc+b
        lin_b = sl[::-1, :].reshape(-1)                       # i = k*Bc+b, t=S-1-k
        # interleave 128-row blocks: [bwd m, fwd m] per block pair
        fb = lin_b.reshape(-1, 128)                           # [128 blocks, 128]
        ff = lin_f.reshape(-1, 128)
        lin_c = np.stack([fb, ff], axis=1).reshape(-1)        # [2*COLS]
        in_maps.append({
            "emit": emit,
            "idxc": _wrap_idx(lin_c),
            "w": W,
            "ident": ident,
            "piext": pi_ext,
        })

    res = run_bass_kernel_spmd(nc, in_maps, core_ids=list(range(NCORES)))
    LAST_RESULTS = res

    alpha = np.empty((S, B, Z), np.float32)
    beta = np.empty((S, B, Z), np.float32)
    post = np.empty((S, B, Z), np.float32)
    for c in range(NCORES):
        r = res.results[c]
        bs = slice(c * Bc, (c + 1) * Bc)
        alpha[:, bs, :] = r["alpha"].reshape(Z, S, Bc).transpose(1, 2, 0)
        beta[:, bs, :] = r["beta"].reshape(Z, S, Bc).transpose(1, 2, 0)
        post[:, bs, :] = r["post"].reshape(Z, S, Bc).transpose(1, 2, 0)
    return alpha, beta, post


# revision 8
# speedup vs baseline: 1.1853x; 1.1125x over previous
"""HMM forward-backward (batch=256, seq=512, Z=64) on 8 Trainium2 NeuronCores.

Strategy (data parallel over batch, 32 batch elements per core):
  - Emission rows e[t,b,:] = emit[input[t,b]] are gathered on-device with
    dma_gather (int16 indices), landing in natural [row=(t,b), Z] layout,
    then PE-transposed to a [Z, (t,b)] layout ("E2").
  - Forward and backward recursions are merged into ONE 128-contraction
    matmul per step with a block-diagonal stationary matrix
    W = diag(T, T^T):  state = [v_{S-1-q} (rows 0:64) ; alpha_q (rows 64:128)]
    per column group q.  One DVE multiply with the E2 column produces the
    next state column.  beta_{S-2-j} is the top PSUM half before the
    multiply and is copied off by the Scalar engine.
  - posterior = (alpha*beta) normalized per (t,b) over Z: Z lives on the
    partition dim, so the column sums use a ones-vector matmul, then
    reciprocal + gpsimd partition_broadcast + DVE multiplies.
  - Outputs are produced in [Z, t*32+b] layout per core; the host
    reassembles/transposes to [S, B, Z] (pure numpy layout work).
"""

import sys

for _p in ("/opt/trn_rl_repo", "/root/.axon_site/_ro/trn_rl_repo"):
    if _p not in sys.path:
        sys.path.append(_p)

import numpy as np

import concourse.bacc as bacc
import concourse.mybir as mybir
from concourse.bass_utils import run_bass_kernel_spmd
from concourse.tile import TileContext

S = 512          # sequence length
B = 256          # total batch
Z = 64           # hidden states
X = 10000        # emission vocab
NCORES = 8
Bc = B // NCORES           # batch per core = 32
COLS = S * Bc              # 16384 state columns per core
CH = 64                    # timesteps per gather/E2 chunk
CCOLS = CH * Bc            # 2048 columns per chunk
NCH = S // CH              # 8 chunks
NBLK = CCOLS // 128        # 16 transpose blocks (of 4 timesteps) per chunk
PCOLS = 1024               # posterior chunk columns (32 timesteps)
NPCH = COLS // PCOLS       # 16 posterior chunks

F32 = mybir.dt.float32
MUL = mybir.AluOpType.mult

_CACHE = {}
LAST_RESULTS = None


def _build_nc():
    nc = bacc.Bacc("TRN2", target_bir_lowering=False, debug=False,
                   num_devices=NCORES)

    emit_d = nc.dram_tensor("emit", [X, Z], F32, kind="ExternalInput")
    idxc_d = nc.dram_tensor("idxc", [128, 2 * COLS // 16], mybir.dt.int16,
                            kind="ExternalInput")
    w_d = nc.dram_tensor("w", [128, 128], F32, kind="ExternalInput")
    id_d = nc.dram_tensor("ident", [128, 128], F32, kind="ExternalInput")
    pi_d = nc.dram_tensor("piext", [128, 1], F32, kind="ExternalInput")

    alpha_d = nc.dram_tensor("alpha", [64, COLS], F32, kind="ExternalOutput")
    beta_d = nc.dram_tensor("beta", [64, COLS], F32, kind="ExternalOutput")
    post_d = nc.dram_tensor("post", [64, COLS], F32, kind="ExternalOutput")

    with TileContext(nc) as tc:
        with (
            tc.tile_pool(name="const", bufs=1) as constp,
            tc.tile_pool(name="state", bufs=1) as statep,
            tc.tile_pool(name="betap", bufs=1) as betapp,
            tc.tile_pool(name="e2", bufs=2) as e2p,
            tc.tile_pool(name="gst", bufs=3) as gstp,
            tc.tile_pool(name="ab", bufs=2) as abp,
            tc.tile_pool(name="bc", bufs=2) as bcp,
            tc.tile_pool(name="po", bufs=2) as pop,
            tc.tile_pool(name="rec", bufs=2) as recp,
            tc.tile_pool(name="mm", bufs=4, space="PSUM") as mmp,
            tc.tile_pool(name="tr", bufs=4, space="PSUM") as trp,
        ):
            # ---- constants ----
            idxc_t = constp.tile([128, 2 * COLS // 16], mybir.dt.int16,
                                 tag="idxc")
            w_t = constp.tile([128, 128], F32, tag="w")
            id_t = constp.tile([128, 128], F32, tag="id")
            pi_t = constp.tile([128, 1], F32, tag="pi")
            nc.sync.dma_start(idxc_t[:], idxc_d[:])
            nc.sync.dma_start(w_t[:], w_d[:])
            nc.sync.dma_start(id_t[:], id_d[:])
            nc.sync.dma_start(pi_t[:], pi_d[:])

            state = statep.tile([128, COLS], F32, tag="state")
            betap = betapp.tile([128, COLS], F32, tag="beta")  # rows 64:128

            gtiles = {}   # (chunk, which 0=bwd/top 1=fwd/bottom) -> tile
            e2tiles = {}  # chunk -> tile

            # dma_gather is limited to ~1024 indices per instruction.
            # One interleaved gather per chunk: even blocks = bwd rows,
            # odd blocks = fwd rows -> [128, 2*NBLK, 64] staging.
            GI = 1024
            GSPLIT = 2 * CCOLS // GI  # 4 gathers per chunk

            def issue_gather(c):
                g = gstp.tile([128, 2 * NBLK, Z], F32, tag="g", name=f"g_{c}")
                for h in range(GSPLIT):
                    nb = GI // 128
                    nc.gpsimd.dma_gather(
                        g[:, h * nb:(h + 1) * nb, :], emit_d[:],
                        idxc_t[:, c * (2 * CCOLS // 16) + h * (GI // 16):
                               c * (2 * CCOLS // 16) + (h + 1) * (GI // 16)],
                        GI, GI, Z)
                gtiles[c] = g

            def transpose_pair(c, m):
                if m == 0:
                    e2tiles[c] = e2p.tile([128, CCOLS], F32, tag="e2", name=f"e2_{c}")
                e2 = e2tiles[c]
                pA = trp.tile([128, 128], F32, tag="tr")
                gv = gtiles[c][:, 2 * m:2 * m + 2, :]
                nc.tensor.transpose(pA[:], gv.rearrange("p a b -> p (a b)"),
                                    id_t[:])
                cs = slice(m * 128, (m + 1) * 128)
                nc.scalar.copy(e2[:, cs], pA[:])

            # ---- prologue ----
            issue_gather(0)
            issue_gather(1)
            for m in range(NBLK):
                transpose_pair(0, m)
            # state col 0 = E2 col 0 * [ones; pi]
            nc.vector.tensor_scalar(state[:, 0:Bc], e2tiles[0][:, 0:Bc],
                                    pi_t[:, 0:1], None, MUL)
            # beta[S-1] = 1
            nc.vector.memset(betap[64:128, (S - 1) * Bc:S * Bc], 1.0)

            # ---- posterior machinery: sliced ops so they fit engine idle
            # windows inside the scan without stretching the chain ----
            PSL = 256                       # posterior slice columns
            NSL = PCOLS // PSL              # 4 slices per chunk

            def post_ops(p):
                """Closures computing posterior chunk p, in dependency order,
                each small enough to hide in per-step engine idle time."""
                cs = slice(p * PCOLS, (p + 1) * PCOLS)
                ctx = {}
                ops = []

                def mk_ab(k):
                    def fn():
                        if k == 0:
                            ctx["ab"] = abp.tile([64, PCOLS], F32, tag="ab",
                                                 name=f"ab_{p}")
                        s = slice(p * PCOLS + k * PSL, p * PCOLS + (k + 1) * PSL)
                        nc.vector.tensor_tensor(
                            ctx["ab"][:, k * PSL:(k + 1) * PSL],
                            state[64:128, s], betap[64:128, s], MUL)
                    return fn

                def mk_sum(k):
                    def fn():
                        if k == 0:
                            ctx["rec"] = recp.tile([1, PCOLS], F32, tag="rec",
                                                   name=f"rec_{p}")
                        pssum = trp.tile([1, PSL], F32, tag="tr",
                                         name=f"pss_{p}_{k}")
                        nc.tensor.matmul(pssum[:], pi_t[0:64, 0:1],
                                         ctx["ab"][:, k * PSL:(k + 1) * PSL])
                        nc.vector.reciprocal(
                            ctx["rec"][:, k * PSL:(k + 1) * PSL], pssum[:])
                    return fn

                def mk_bcast():
                    def fn():
                        ctx["bc"] = bcp.tile([64, PCOLS], F32, tag="bc",
                                             name=f"bc_{p}")
                        nc.gpsimd.partition_broadcast(ctx["bc"][:],
                                                      ctx["rec"][:, :])
                    return fn

                def mk_pm(k):
                    def fn():
                        if k == 0:
                            ctx["po"] = pop.tile([64, PCOLS], F32, tag="po",
                                                 name=f"po_{p}")
                        nc.vector.tensor_tensor(
                            ctx["po"][:, k * PSL:(k + 1) * PSL],
                            ctx["ab"][:, k * PSL:(k + 1) * PSL],
                            ctx["bc"][:, k * PSL:(k + 1) * PSL], MUL)
                        if k == NSL - 1:
                            nc.sync.dma_start(post_d[:, cs], ctx["po"][:])
                    return fn

                for k in range(NSL):
                    ops.append(mk_ab(k))
                for k in range(NSL):
                    ops.append(mk_sum(k))
                ops.append(mk_bcast())
                for k in range(NSL):
                    ops.append(mk_pm(k))
                return ops

            # chunks whose alpha+beta are ready mid-scan: p=8..14 inline,
            # two ops every step starting 2 steps after alpha lands
            TAIL_PCH = [15, 0]
            POST_SCHED = {}
            for p in range(8, 15):          # alpha-bound: ready after 32p+31
                for i, fn in enumerate(post_ops(p)):
                    POST_SCHED.setdefault(32 * p + 33 + 2 * i, []).append(fn)
            for p in range(1, 8):           # beta-bound: ready after 510-32p
                for i, fn in enumerate(post_ops(p)):
                    POST_SCHED.setdefault(512 - 32 * p + 2 * i, []).append(fn)

            # transpose emission schedule: pair m of chunk c+1 is emitted
            # inside chunk c at step offset TR_SLOTS[m]
            TR_SLOTS = {4 + 3 * m: m for m in range(NBLK)}

            # ---- merged forward/backward scan ----
            for j in range(S - 1):
                c, off = j // CH, j % CH
                if off == 0 and c + 2 < NCH:
                    issue_gather(c + 2)
                if off in TR_SLOTS and c + 1 < NCH:
                    transpose_pair(c + 1, TR_SLOTS[off])

                ps = mmp.tile([128, Bc], F32, tag="mm")
                nc.tensor.matmul(ps[:], w_t[:], state[:, j * Bc:(j + 1) * Bc])
                q = j + 1
                nc.vector.tensor_tensor(
                    state[:, q * Bc:(q + 1) * Bc], ps[:],
                    e2tiles[q // CH][:, (q % CH) * Bc:((q % CH) + 1) * Bc],
                    MUL)
                tb = S - 2 - j
                nc.scalar.copy(betap[64:128, tb * Bc:(tb + 1) * Bc],
                               ps[0:64, :])
                # stream completed output chunks out under the scan
                if off == CH - 1 and c < NCH - 1:       # alpha chunk c done
                    acs = slice(c * CCOLS, (c + 1) * CCOLS)
                    nc.sync.dma_start(alpha_d[:, acs], state[64:128, acs])
                bc_ = (S - 1 - j) // CH                 # beta chunk bc_ done when j == 511-64*bc_
                if bc_ >= 1 and j == (S - 1) - CH * bc_ and bc_ <= NCH - 1:
                    bcs = slice(bc_ * CCOLS, (bc_ + 1) * CCOLS)
                    nc.sync.dma_start(beta_d[:, bcs], betap[64:128, bcs])
                for fn in POST_SCHED.get(j, []):
                    fn()

            # ---- outputs: alpha/beta straight out, posterior normalized ----
            cs = slice((NCH - 1) * CCOLS, NCH * CCOLS)
            nc.sync.dma_start(alpha_d[:, cs], state[64:128, cs])
            cs = slice(0, CCOLS)
            nc.sync.dma_start(beta_d[:, cs], betap[64:128, cs])

            for p in TAIL_PCH:
                for fn in post_ops(p):
                    fn()

    nc.finalize()
    return nc


def _wrap_idx(lin):
    """Linear index list -> [128, N//16] int16 gather-index layout
    (position i at [i % 16, i // 16], replicated over partition groups)."""
    n = lin.shape[0]
    w = lin.reshape(n // 16, 16).T.astype(np.int16)   # [16, n//16]
    return np.tile(w, (8, 1))


def kernel(input, T, pi, emit):
    global LAST_RESULTS
    input = np.asarray(input)
    T = np.asarray(T, dtype=np.float32)
    pi = np.asarray(pi, dtype=np.float32)
    emit = np.asarray(emit, dtype=np.float32)

    if "nc" not in _CACHE:
        _CACHE["nc"] = _build_nc()
    nc = _CACHE["nc"]

    W = np.zeros((128, 128), np.float32)
    W[:64, :64] = T          # backward block: out_top = T^T @ v
    W[64:, 64:] = T.T        # forward block:  out_bot = T @ alpha
    pi_ext = np.ones((128, 1), np.float32)
    pi_ext[64:, 0] = pi
    ident = np.eye(128, dtype=np.float32)

    in_maps = []
    for c in range(NCORES):
        sl = input[:, c * Bc:(c + 1) * Bc].astype(np.int64)   # [S, Bc]
        lin_f = sl.reshape(-1)                                # i = t*Bc+b
        lin_b = sl[::-1, :].reshape(-1)                       # i = k*Bc+b, t=S-1-k
        # interleave 128-row blocks: [bwd m, fwd m] per block pair
        fb = lin_b.reshape(-1, 128)                           # [128 blocks, 128]
        ff = lin_f.reshape(-1, 128)
        lin_c = np.stack([fb, ff], axis=1).reshape(-1)        # [2*COLS]
        in_maps.append({
            "emit": emit,
            "idxc": _wrap_idx(lin_c),
            "w": W,
            "ident": ident,
            "piext": pi_ext,
        })

    res = run_bass_kernel_spmd(nc, in_maps, core_ids=list(range(NCORES)))
    LAST_RESULTS = res

    alpha = np.empty((S, B, Z), np.float32)
    beta = np.empty((S, B, Z), np.float32)
    post = np.empty((S, B, Z), np.float32)
    for c in range(NCORES):
        r = res.results[c]
        bs = slice(c * Bc, (c + 1) * Bc)
        alpha[:, bs, :] = r["alpha"].reshape(Z, S, Bc).transpose(1, 2, 0)
        beta[:, bs, :] = r["beta"].reshape(Z, S, Bc).transpose(1, 2, 0)
        post[:, bs, :] = r["post"].reshape(Z, S, Bc).transpose(1, 2, 0)
    return alpha, beta, post
